# revision 3
# baseline (speedup 1.0000x reference)
"""Trainium2 Bass kernel for causal multi-head attention (dense transformer block).

Problem (hardcoded): x [2, 2048, 1024], 16 heads x 64 dh, causal attention,
fp32. Sharding: 8 cores = 2 batches x 4 head-groups. Each core computes 4
heads for one batch and a partial output projection [2048, 1024]; the host
sums the 4 partials per batch and adds b_O.

Everything on-device is computed in "transposed" orientation so no transposes
are needed anywhere:
  x^T (host-pretransposed)  ->  Q^T, K^T [dh, s] and V [s, dh] via matmuls
  S^T[k, q] = K Q^T         ->  P^T = exp(S^T / 8) (causal-masked)
  Z^T[dh, q] = V^T P^T      ->  normalized by column sums (ones-matmul)
  O[s, :]   = (Z^T)^T W_O   (Z^T is directly the lhsT of the O-projection)

Heads are processed in pairs: QK^T packs 2 heads in row-groups (0-63 / 64-127)
of the PE array, PV packs 2 heads in column-groups -- both run concurrently.
"""

import os
from contextlib import ExitStack

import numpy as np

import concourse.bass as bass
import concourse.tile as tile
from concourse import bacc, mybir
from concourse.bass_utils import run_bass_kernel_spmd

# problem constants
B, S, DM, H, DH = 2, 2048, 1024, 16, 64
P = 128          # partitions
QB = 512         # q block (matmul moving free dim)
NKT = S // P     # 16 k tiles
NQB = S // QB    # 4 q blocks
NDM = DM // P    # 8 d_model tiles
HPC = 4          # heads per core
NCORES = 8

F32 = mybir.dt.float32
BF16 = mybir.dt.bfloat16

# matmul input dtype mode: "fp32" | "fp32r" | "bf16"
MM_DTYPE = os.environ.get("ATTN_MM_DTYPE", "fp32r")

_PROGRAM_CACHE = {}
LAST_RESULTS = None  # BassKernelResults of the most recent run (for test.py)


def _mm(nc, out, lhsT, rhs, start, stop, fp32r, skip=False):
    if fp32r:
        lhsT = lhsT.bitcast(mybir.dt.float32r)
        rhs = rhs.bitcast(mybir.dt.float32r)
    # skip_group_check: the sim's psum-group tracker doesn't distinguish
    # partition ranges; our concurrent groups in one bank are partition-disjoint
    # (rows 0-63 vs 64-127), which the per-partition zeroing model handles.
    nc.tensor.matmul(out, lhsT, rhs, start=start, stop=stop, skip_group_check=skip)


def build_program(mm_dtype=MM_DTYPE):
    """Build the single-core SPMD Bass program (same program on all 8 cores)."""
    if mm_dtype in _PROGRAM_CACHE:
        return _PROGRAM_CACHE[mm_dtype]

    fp32r = mm_dtype == "fp32r"
    DT = BF16 if mm_dtype == "bf16" else F32

    nc = bacc.Bacc(
        "TRN2", target_bir_lowering=False, debug=False, num_devices=NCORES
    )

    # ---- DRAM I/O (per-core shards, prearranged on host) ----
    xT_d = nc.dram_tensor("xT", [DM, S], DT, kind="ExternalInput")
    wq_d = nc.dram_tensor("wq", [DM, HPC * DH], DT, kind="ExternalInput")
    wk_d = nc.dram_tensor("wk", [DM, HPC * DH], DT, kind="ExternalInput")
    wv_d = nc.dram_tensor("wv", [DM, HPC * DH], DT, kind="ExternalInput")
    wo_d = nc.dram_tensor("wo", [HPC * DH, DM], DT, kind="ExternalInput")
    bq_d = nc.dram_tensor("bq", [2, P], F32, kind="ExternalInput")
    bk_d = nc.dram_tensor("bk", [2, P], F32, kind="ExternalInput")
    bv_d = nc.dram_tensor("bv", [P, HPC * DH], F32, kind="ExternalInput")
    diag_d = nc.dram_tensor("diag", [P, P], DT, kind="ExternalInput")
    out_d = nc.dram_tensor("out", [S, DM], F32, kind="ExternalOutput")

    with tile.TileContext(nc) as tc, ExitStack() as ctx:
        const = ctx.enter_context(tc.tile_pool(name="const", bufs=1))
        persist = ctx.enter_context(tc.tile_pool(name="persist", bufs=1))

        # ---- constants ----
        ones64 = const.tile([P, 64], DT, name="ones64", tag="ones64")
        nc.gpsimd.memset(ones64[:], 1.0)
        diag_sb = const.tile([P, P], DT, name="diag_sb", tag="diag")
        nc.sync.dma_start(out=diag_sb[:], in_=diag_d[:, :])
        bq_sb = const.tile([P, 2], F32, name="bq_sb", tag="bq")
        bk_sb = const.tile([P, 2], F32, name="bk_sb", tag="bk")
        for p in range(2):
            nc.sync.dma_start(out=bq_sb[:, p : p + 1], in_=bq_d[p : p + 1, :])
            nc.sync.dma_start(out=bk_sb[:, p : p + 1], in_=bk_d[p : p + 1, :])
        bv_sb = const.tile([P, HPC * DH], F32, name="bv_sb", tag="bv")
        nc.sync.dma_start(out=bv_sb[:], in_=bv_d[:, :])

        # ---- persistent activations ----
        qt_sb = [
            persist.tile([P, S], DT, name=f"qt{p}", tag=f"qt{p}") for p in range(2)
        ]
        kt_sb = [
            persist.tile([P, S], DT, name=f"kt{p}", tag=f"kt{p}") for p in range(2)
        ]
        v_sb = [
            persist.tile([P, NKT, P], DT, name=f"v{p}", tag=f"v{p}")
            for p in range(2)
        ]
        zt_sb = [
            persist.tile([P, S], DT, name=f"zt{p}", tag=f"zt{p}") for p in range(2)
        ]
        wo_sb = persist.tile([P, 2, DM], DT, name="wo_sb", tag="wo")
        for p in range(2):
            nc.sync.dma_start(out=wo_sb[:, p, :], in_=wo_d[p * P : (p + 1) * P, :])

        # ================= phase 1: QKV projections =================
        with ExitStack() as ph1:
            xw = ph1.enter_context(tc.tile_pool(name="xw", bufs=1))
            pp1 = ph1.enter_context(
                tc.tile_pool(name="pp1", bufs=3, space="PSUM")
            )

            xt_sb = xw.tile([P, NDM, S], DT, name="xt_sb", tag="xt")
            for t in range(NDM):
                nc.sync.dma_start(
                    out=xt_sb[:, t, :], in_=xT_d[t * P : (t + 1) * P, :]
                )
            w_sb = {}
            for wname, wd in (("wq", wq_d), ("wk", wk_d), ("wv", wv_d)):
                w_sb[wname] = xw.tile(
                    [P, NDM, HPC * DH], DT, name=f"{wname}_sb", tag=wname
                )
                for t in range(NDM):
                    nc.sync.dma_start(
                        out=w_sb[wname][:, t, :], in_=wd[t * P : (t + 1) * P, :]
                    )

            # Q^T and K^T: [dh-pair (128), seq] ; pair rows 0-63 = head 2p,
            # 64-127 = head 2p+1
            for dst, wname, bias in (
                (qt_sb, "wq", bq_sb),
                (kt_sb, "wk", bk_sb),
            ):
                for p in range(2):
                    for ch in range(NQB):
                        qp = pp1.tile([P, QB], F32, name="qp", tag="qp")
                        for t in range(NDM):
                            _mm(
                                nc,
                                qp[:],
                                w_sb[wname][:, t, p * P : (p + 1) * P],
                                xt_sb[:, t, ch * QB : (ch + 1) * QB],
                                start=(t == 0),
                                stop=(t == NDM - 1),
                                fp32r=fp32r,
                            )
                        nc.vector.tensor_scalar_add(
                            dst[p][:, ch * QB : (ch + 1) * QB],
                            qp[:],
                            bias[:, p : p + 1],
                        )

            # V: [seq, head-pair dh] per 128-row tile
            for st in range(NKT):
                vp = pp1.tile([P, HPC * DH], F32, name="vp", tag="vp")
                for t in range(NDM):
                    _mm(
                        nc,
                        vp[:],
                        xt_sb[:, t, st * P : (st + 1) * P],
                        w_sb["wv"][:, t, :],
                        start=(t == 0),
                        stop=(t == NDM - 1),
                        fp32r=fp32r,
                    )
                for p in range(2):
                    nc.vector.tensor_add(
                        v_sb[p][:, st, :],
                        vp[:, p * P : (p + 1) * P],
                        bv_sb[:, p * P : (p + 1) * P],
                    )

        # ================= phase 2: attention =================
        with ExitStack() as ph2:
            sp = ph2.enter_context(tc.tile_pool(name="sp", bufs=3, space="PSUM"))
            zp = ph2.enter_context(tc.tile_pool(name="zp", bufs=1, space="PSUM"))
            dp = ph2.enter_context(tc.tile_pool(name="dp", bufs=1, space="PSUM"))
            ppool = ph2.enter_context(tc.tile_pool(name="ppool", bufs=3))
            cspool = ph2.enter_context(tc.tile_pool(name="cspool", bufs=4))
            bcpool = ph2.enter_context(tc.tile_pool(name="bcpool", bufs=2))

            for p in range(2):
                for qb in range(NQB):
                    nk = (qb + 1) * (QB // P)  # k tiles in causal range
                    q0 = qb * QB
                    zps = zp.tile([P, QB], F32, name="zps", tag="z")
                    csA = cspool.tile([P, 2, QB], F32, name="csA", tag="cs")
                    csB = cspool.tile([P, 2, QB], F32, name="csB", tag="cs")
                    nc.gpsimd.memset(csA[:], 0.0)
                    nc.gpsimd.memset(csB[:], 0.0)

                    for kg in range(nk // 2):
                        sA = sp.tile([P, 2, QB], F32, name="sA", tag="s")
                        sB = sp.tile([P, 2, QB], F32, name="sB", tag="s")
                        for j in range(2):
                            kt = kg * 2 + j
                            # 2 heads row-packed: A in partitions 0-63,
                            # B in 64-127, concurrent on the PE array
                            _mm(
                                nc,
                                sA[:, j, :],
                                kt_sb[p][0:64, kt * P : (kt + 1) * P],
                                qt_sb[p][0:64, q0 : q0 + QB],
                                start=True,
                                stop=True,
                                fp32r=fp32r,
                            )
                            _mm(
                                nc,
                                sB[:, j, :],
                                kt_sb[p][64:P, kt * P : (kt + 1) * P],
                                qt_sb[p][64:P, q0 : q0 + QB],
                                start=True,
                                stop=True,
                                fp32r=fp32r,
                            )
                        pA = ppool.tile([P, 2, QB], DT, name="pA", tag="pt")
                        pB = ppool.tile([P, 2, QB], DT, name="pB", tag="pt")
                        # exp(S/sqrt(dh)); scale folded into ACT
                        nc.scalar.activation(
                            pA[:], sA[:], mybir.ActivationFunctionType.Exp,
                            scale=0.125,
                        )
                        nc.scalar.activation(
                            pB[:], sB[:], mybir.ActivationFunctionType.Exp,
                            scale=0.125,
                        )
                        # causal mask on diagonal-band tiles
                        for j in range(2):
                            kt = kg * 2 + j
                            off = kt * P - q0
                            if off >= 0:
                                for px in (pA, pB):
                                    if off > 0:
                                        nc.vector.memset(px[:, j, 0:off], 0.0)
                                    nc.vector.tensor_mul(
                                        px[:, j, off : off + P],
                                        px[:, j, off : off + P],
                                        diag_sb[:],
                                    )
                        # running column-sums (softmax denominators)
                        nc.vector.tensor_add(csA[:], csA[:], pA[:])
                        nc.vector.tensor_add(csB[:], csB[:], pB[:])
                        # PV: 2 heads column-packed, accumulated over k
                        for j in range(2):
                            kt = kg * 2 + j
                            _mm(
                                nc,
                                zps[0:64, :],
                                v_sb[p][:, kt, 0:64],
                                pA[:, j, :],
                                start=(kt == 0),
                                stop=(kt == nk - 1),
                                fp32r=fp32r,
                                skip=True,
                            )
                            _mm(
                                nc,
                                zps[64:P, :],
                                v_sb[p][:, kt, 64:P],
                                pB[:, j, :],
                                start=(kt == 0),
                                stop=(kt == nk - 1),
                                fp32r=fp32r,
                                skip=True,
                            )

                    # denominators: ones-matmul sums partitions of the column
                    # sums AND broadcasts across 64 rows in one shot
                    dnb = dp.tile([P, QB], F32, name="dnb", tag="d")
                    for jj in range(2):
                        _mm(
                            nc, dnb[0:64, :], ones64[:], csA[:, jj, :],
                            start=(jj == 0), stop=(jj == 1), fp32r=fp32r,
                            skip=True,
                        )
                    for jj in range(2):
                        _mm(
                            nc, dnb[64:P, :], ones64[:], csB[:, jj, :],
                            start=(jj == 0), stop=(jj == 1), fp32r=fp32r,
                            skip=True,
                        )
                    bcr = bcpool.tile([P, QB], F32, name="bcr", tag="bcr")
                    nc.vector.reciprocal(bcr[:], dnb[:])
                    nc.vector.tensor_mul(
                        zt_sb[p][:, q0 : q0 + QB], zps[:], bcr[:]
                    )

        # ================= phase 3: output projection =================
        with ExitStack() as ph3:
            op = ph3.enter_context(tc.tile_pool(name="op", bufs=4, space="PSUM"))
            ost = ph3.enter_context(tc.tile_pool(name="ost", bufs=4))
            for st in range(NKT):
                for nn in range(2):
                    ops = op.tile([P, QB], F32, name="ops", tag="o")
                    for p in range(2):
                        _mm(
                            nc,
                            ops[:],
                            zt_sb[p][:, st * P : (st + 1) * P],
                            wo_sb[:, p, nn * QB : (nn + 1) * QB],
                            start=(p == 0),
                            stop=(p == 1),
                            fp32r=fp32r,
                        )
                    ot = ost.tile([P, QB], F32, name="ot", tag="ot")
                    nc.vector.tensor_copy(ot[:], ops[:])
                    nc.sync.dma_start(
                        out=out_d[st * P : (st + 1) * P, nn * QB : (nn + 1) * QB],
                        in_=ot[:],
                    )

    nc.compile()
    _PROGRAM_CACHE[mm_dtype] = nc
    return nc


def make_in_maps(
    normalized_resid_pre, W_Q, W_K, W_V, W_O, b_Q, b_K, b_V, b_O,
    mm_dtype=MM_DTYPE,
):
    """Shard + prearrange the full inputs into per-core input maps."""
    np_dt = np.float32 if mm_dtype != "bf16" else np.dtype("bfloat16")
    import ml_dtypes  # noqa: F401  (registers bfloat16 with numpy)

    x = np.asarray(normalized_resid_pre, dtype=np.float32)
    W_Q = np.asarray(W_Q, dtype=np.float32)
    W_K = np.asarray(W_K, dtype=np.float32)
    W_V = np.asarray(W_V, dtype=np.float32)
    W_O = np.asarray(W_O, dtype=np.float32)
    b_Q = np.asarray(b_Q, dtype=np.float32)
    b_K = np.asarray(b_K, dtype=np.float32)
    b_V = np.asarray(b_V, dtype=np.float32)

    xT = [np.ascontiguousarray(x[b].T).astype(np_dt) for b in range(B)]
    diag = np.triu(np.ones((P, P), dtype=np.float32), k=0).astype(np_dt)

    in_maps = []
    for c in range(NCORES):
        b = c // (NCORES // B)
        heads = [HPC * (c % (NCORES // B)) + i for i in range(HPC)]
        wq = np.concatenate([W_Q[h] for h in heads], axis=1).astype(np_dt)
        wk = np.concatenate([W_K[h] for h in heads], axis=1).astype(np_dt)
        wv = np.concatenate([W_V[h] for h in heads], axis=1).astype(np_dt)
        wo = np.concatenate([W_O[h] for h in heads], axis=0).astype(np_dt)
        bq = np.stack(
            [
                np.concatenate([b_Q[heads[0]], b_Q[heads[1]]]),
                np.concatenate([b_Q[heads[2]], b_Q[heads[3]]]),
            ]
        ).astype(np.float32)
        bk = np.stack(
            [
                np.concatenate([b_K[heads[0]], b_K[heads[1]]]),
                np.concatenate([b_K[heads[2]], b_K[heads[3]]]),
            ]
        ).astype(np.float32)
        bv = np.tile(
            np.concatenate([b_V[h] for h in heads])[None, :], (P, 1)
        ).astype(np.float32)
        in_maps.append(
            {
                "xT": np.ascontiguousarray(xT[b]),
                "wq": wq, "wk": wk, "wv": wv, "wo": wo,
                "bq": bq, "bk": bk, "bv": bv,
                "diag": diag,
            }
        )
    return in_maps


def kernel(normalized_resid_pre, W_Q, W_K, W_V, W_O, b_Q, b_K, b_V, b_O):
    global LAST_RESULTS
    nc = build_program()
    in_maps = make_in_maps(
        normalized_resid_pre, W_Q, W_K, W_V, W_O, b_Q, b_K, b_V, b_O
    )
    trace = os.environ.get("ATTN_TRACE", "0") == "1"
    res = run_bass_kernel_spmd(nc, in_maps, list(range(NCORES)), trace=trace)
    LAST_RESULTS = res

    b_O = np.asarray(b_O, dtype=np.float32)
    parts = [np.asarray(res.results[c]["out"], dtype=np.float64) for c in range(NCORES)]
    npc = NCORES // B  # cores per batch
    out = np.stack(
        [sum(parts[b * npc : (b + 1) * npc]) + b_O for b in range(B)]
    )
    return out.astype(np.float32)


# revision 4
# speedup vs baseline: 1.8311x; 1.8311x over previous
"""Trainium2 Bass kernel for causal multi-head attention (dense transformer block).

Problem (hardcoded): x [2, 2048, 1024], 16 heads x 64 dh, causal attention,
fp32. Sharding: 8 cores = 2 batches x 4 head-groups. Each core computes 4
heads for one batch and a partial output projection [2048, 1024]; the host
sums the 4 partials per batch and adds b_O.

Everything on-device is computed in "transposed" orientation so no transposes
are needed anywhere:
  x^T (host-pretransposed)  ->  Q^T, K^T [dh, s] and V [s, dh] via matmuls
  S^T[k, q] = K Q^T         ->  P^T = exp(S^T / 8) (causal-masked)
  Z^T[dh, q] = V^T P^T      ->  normalized by column sums (ones-matmul)
  O[s, :]   = (Z^T)^T W_O   (Z^T is directly the lhsT of the O-projection)

Heads are processed in pairs: QK^T packs 2 heads in row-groups (0-63 / 64-127)
of the PE array, PV packs 2 heads in column-groups -- both run concurrently.
"""

import os
from contextlib import ExitStack

import numpy as np

import concourse.bass as bass
import concourse.tile as tile
from concourse import bacc, mybir
from concourse.bass_utils import run_bass_kernel_spmd

# problem constants
B, S, DM, H, DH = 2, 2048, 1024, 16, 64
P = 128          # partitions
QB = 512         # q block (matmul moving free dim)
NKT = S // P     # 16 k tiles
NQB = S // QB    # 4 q blocks
NDM = DM // P    # 8 d_model tiles
HPC = 4          # heads per core
NCORES = 8

F32 = mybir.dt.float32
BF16 = mybir.dt.bfloat16

# matmul input dtype mode: "fp32" | "fp32r" | "bf16"
MM_DTYPE = os.environ.get("ATTN_MM_DTYPE", "fp32r")

_PROGRAM_CACHE = {}
LAST_RESULTS = None  # BassKernelResults of the most recent run (for test.py)


def _mm(nc, out, lhsT, rhs, start, stop, fp32r=False, skip=False):
    # skip_group_check: the sim's psum-group tracker doesn't distinguish
    # partition ranges; our concurrent groups in one bank are partition-disjoint
    # (rows 0-63 vs 64-127), which the per-partition zeroing model handles.
    nc.tensor.matmul(out, lhsT, rhs, start=start, stop=stop, skip_group_check=skip)


def build_program(mm_dtype=MM_DTYPE):
    """Build the single-core SPMD Bass program (same program on all 8 cores)."""
    if mm_dtype in _PROGRAM_CACHE:
        return _PROGRAM_CACHE[mm_dtype]

    fp32r = mm_dtype == "fp32r"
    DT = {"bf16": BF16, "fp32r": mybir.dt.float32r, "fp32": F32}[mm_dtype]

    nc = bacc.Bacc(
        "TRN2", target_bir_lowering=False, debug=False, num_devices=NCORES
    )

    # ---- DRAM I/O (per-core shards, prearranged on host) ----
    xT_d = nc.dram_tensor("xT", [DM, S], DT, kind="ExternalInput")
    wq_d = nc.dram_tensor("wq", [DM, HPC * DH], DT, kind="ExternalInput")
    wk_d = nc.dram_tensor("wk", [DM, HPC * DH], DT, kind="ExternalInput")
    wv_d = nc.dram_tensor("wv", [DM, HPC * DH], DT, kind="ExternalInput")
    wo_d = nc.dram_tensor("wo", [HPC * DH, DM], DT, kind="ExternalInput")
    bq_d = nc.dram_tensor("bq", [2, P], F32, kind="ExternalInput")
    bk_d = nc.dram_tensor("bk", [2, P], F32, kind="ExternalInput")
    bv_d = nc.dram_tensor("bv", [P, HPC * DH], F32, kind="ExternalInput")
    diag_d = nc.dram_tensor("diag", [P, P], DT, kind="ExternalInput")
    out_d = nc.dram_tensor("out", [S, DM], F32, kind="ExternalOutput")

    with tile.TileContext(nc) as tc, ExitStack() as ctx:
        const = ctx.enter_context(tc.tile_pool(name="const", bufs=1))
        persist = ctx.enter_context(tc.tile_pool(name="persist", bufs=1))

        # ---- constants ----
        ones64 = const.tile([P, 64], F32, name="ones64", tag="ones64")
        nc.gpsimd.memset(ones64[:], 1.0)
        diag_sb = const.tile([P, P], DT, name="diag_sb", tag="diag")
        nc.sync.dma_start(out=diag_sb[:], in_=diag_d[:, :])
        bq_sb = const.tile([P, 2], F32, name="bq_sb", tag="bq")
        bk_sb = const.tile([P, 2], F32, name="bk_sb", tag="bk")
        for p in range(2):
            nc.sync.dma_start(out=bq_sb[:, p : p + 1], in_=bq_d[p : p + 1, :])
            nc.sync.dma_start(out=bk_sb[:, p : p + 1], in_=bk_d[p : p + 1, :])
        bv_sb = const.tile([P, HPC * DH], F32, name="bv_sb", tag="bv")
        nc.sync.dma_start(out=bv_sb[:], in_=bv_d[:, :])

        # ---- persistent activations ----
        qt_sb = [
            persist.tile([P, S], DT, name=f"qt{p}", tag=f"qt{p}") for p in range(2)
        ]
        kt_sb = [
            persist.tile([P, S], DT, name=f"kt{p}", tag=f"kt{p}") for p in range(2)
        ]
        v_sb = [
            persist.tile([P, NKT, P], DT, name=f"v{p}", tag=f"v{p}")
            for p in range(2)
        ]
        zt_sb = [
            persist.tile([P, S], DT, name=f"zt{p}", tag=f"zt{p}") for p in range(2)
        ]
        wo_sb = persist.tile([P, 2, DM], DT, name="wo_sb", tag="wo")
        for p in range(2):
            nc.sync.dma_start(out=wo_sb[:, p, :], in_=wo_d[p * P : (p + 1) * P, :])

        # ================= phase 1: QKV projections =================
        with ExitStack() as ph1:
            xw = ph1.enter_context(tc.tile_pool(name="xw", bufs=1))
            pp1 = ph1.enter_context(
                tc.tile_pool(name="pp1", bufs=3, space="PSUM")
            )

            xt_sb = xw.tile([P, NDM, S], DT, name="xt_sb", tag="xt")
            for t in range(NDM):
                nc.sync.dma_start(
                    out=xt_sb[:, t, :], in_=xT_d[t * P : (t + 1) * P, :]
                )
            w_sb = {}
            for wname, wd in (("wq", wq_d), ("wk", wk_d), ("wv", wv_d)):
                w_sb[wname] = xw.tile(
                    [P, NDM, HPC * DH], DT, name=f"{wname}_sb", tag=wname
                )
                for t in range(NDM):
                    nc.sync.dma_start(
                        out=w_sb[wname][:, t, :], in_=wd[t * P : (t + 1) * P, :]
                    )

            # Q^T and K^T: [dh-pair (128), seq] ; pair rows 0-63 = head 2p,
            # 64-127 = head 2p+1
            for dst, wname, bias in (
                (qt_sb, "wq", bq_sb),
                (kt_sb, "wk", bk_sb),
            ):
                for p in range(2):
                    for ch in range(NQB):
                        qp = pp1.tile([P, QB], F32, name="qp", tag="qp")
                        for t in range(NDM):
                            _mm(
                                nc,
                                qp[:],
                                w_sb[wname][:, t, p * P : (p + 1) * P],
                                xt_sb[:, t, ch * QB : (ch + 1) * QB],
                                start=(t == 0),
                                stop=(t == NDM - 1),
                                fp32r=fp32r,
                            )
                        nc.vector.tensor_scalar_add(
                            dst[p][:, ch * QB : (ch + 1) * QB],
                            qp[:],
                            bias[:, p : p + 1],
                        )

            # V: [seq, head-pair dh] per 128-row tile
            for st in range(NKT):
                vp = pp1.tile([P, HPC * DH], F32, name="vp", tag="vp")
                for t in range(NDM):
                    _mm(
                        nc,
                        vp[:],
                        xt_sb[:, t, st * P : (st + 1) * P],
                        w_sb["wv"][:, t, :],
                        start=(t == 0),
                        stop=(t == NDM - 1),
                        fp32r=fp32r,
                    )
                for p in range(2):
                    nc.vector.tensor_add(
                        v_sb[p][:, st, :],
                        vp[:, p * P : (p + 1) * P],
                        bv_sb[:, p * P : (p + 1) * P],
                    )

        # ================= phase 2: attention =================
        with ExitStack() as ph2:
            sp = ph2.enter_context(tc.tile_pool(name="sp", bufs=3, space="PSUM"))
            zp = ph2.enter_context(tc.tile_pool(name="zp", bufs=1, space="PSUM"))
            dp = ph2.enter_context(tc.tile_pool(name="dp", bufs=1, space="PSUM"))
            ppool = ph2.enter_context(tc.tile_pool(name="ppool", bufs=3))
            cspool = ph2.enter_context(tc.tile_pool(name="cspool", bufs=4))
            bcpool = ph2.enter_context(tc.tile_pool(name="bcpool", bufs=2))

            for p in range(2):
                for qb in range(NQB):
                    nk = (qb + 1) * (QB // P)  # k tiles in causal range
                    q0 = qb * QB
                    zps = zp.tile([P, QB], F32, name="zps", tag="z")
                    csA = cspool.tile([P, 2, QB], F32, name="csA", tag="cs")
                    csB = cspool.tile([P, 2, QB], F32, name="csB", tag="cs")
                    nc.gpsimd.memset(csA[:], 0.0)
                    nc.gpsimd.memset(csB[:], 0.0)

                    for kg in range(nk // 2):
                        sA = sp.tile([P, 2, QB], F32, name="sA", tag="s")
                        sB = sp.tile([P, 2, QB], F32, name="sB", tag="s")
                        for j in range(2):
                            kt = kg * 2 + j
                            # 2 heads row-packed: A in partitions 0-63,
                            # B in 64-127, concurrent on the PE array
                            _mm(
                                nc,
                                sA[:, j, :],
                                kt_sb[p][0:64, kt * P : (kt + 1) * P],
                                qt_sb[p][0:64, q0 : q0 + QB],
                                start=True,
                                stop=True,
                                fp32r=fp32r,
                            )
                            _mm(
                                nc,
                                sB[:, j, :],
                                kt_sb[p][64:P, kt * P : (kt + 1) * P],
                                qt_sb[p][64:P, q0 : q0 + QB],
                                start=True,
                                stop=True,
                                fp32r=fp32r,
                            )
                        pA = ppool.tile([P, 2, QB], DT, name="pA", tag="pt")
                        pB = ppool.tile([P, 2, QB], DT, name="pB", tag="pt")
                        # exp(S/sqrt(dh)); scale folded into ACT
                        nc.scalar.activation(
                            pA[:], sA[:], mybir.ActivationFunctionType.Exp,
                            scale=0.125,
                        )
                        nc.scalar.activation(
                            pB[:], sB[:], mybir.ActivationFunctionType.Exp,
                            scale=0.125,
                        )
                        # causal mask on diagonal-band tiles
                        for j in range(2):
                            kt = kg * 2 + j
                            off = kt * P - q0
                            if off >= 0:
                                for px in (pA, pB):
                                    if off > 0:
                                        nc.vector.memset(px[:, j, 0:off], 0.0)
                                    nc.vector.tensor_mul(
                                        px[:, j, off : off + P],
                                        px[:, j, off : off + P],
                                        diag_sb[:],
                                    )
                        # running column-sums (softmax denominators)
                        nc.vector.tensor_add(csA[:], csA[:], pA[:])
                        nc.vector.tensor_add(csB[:], csB[:], pB[:])
                        # PV: 2 heads column-packed, accumulated over k
                        for j in range(2):
                            kt = kg * 2 + j
                            _mm(
                                nc,
                                zps[0:64, :],
                                v_sb[p][:, kt, 0:64],
                                pA[:, j, :],
                                start=(kt == 0),
                                stop=(kt == nk - 1),
                                fp32r=fp32r,
                                skip=True,
                            )
                            _mm(
                                nc,
                                zps[64:P, :],
                                v_sb[p][:, kt, 64:P],
                                pB[:, j, :],
                                start=(kt == 0),
                                stop=(kt == nk - 1),
                                fp32r=fp32r,
                                skip=True,
                            )

                    # denominators: ones-matmul sums partitions of the column
                    # sums AND broadcasts across 64 rows in one shot
                    dnb = dp.tile([P, QB], F32, name="dnb", tag="d")
                    for jj in range(2):
                        _mm(
                            nc, dnb[0:64, :], ones64[:], csA[:, jj, :],
                            start=(jj == 0), stop=(jj == 1), fp32r=fp32r,
                            skip=True,
                        )
                    for jj in range(2):
                        _mm(
                            nc, dnb[64:P, :], ones64[:], csB[:, jj, :],
                            start=(jj == 0), stop=(jj == 1), fp32r=fp32r,
                            skip=True,
                        )
                    bcr = bcpool.tile([P, QB], F32, name="bcr", tag="bcr")
                    nc.vector.reciprocal(bcr[:], dnb[:])
                    nc.vector.tensor_mul(
                        zt_sb[p][:, q0 : q0 + QB], zps[:], bcr[:]
                    )

        # ================= phase 3: output projection =================
        with ExitStack() as ph3:
            op = ph3.enter_context(tc.tile_pool(name="op", bufs=4, space="PSUM"))
            ost = ph3.enter_context(tc.tile_pool(name="ost", bufs=4))
            for st in range(NKT):
                for nn in range(2):
                    ops = op.tile([P, QB], F32, name="ops", tag="o")
                    for p in range(2):
                        _mm(
                            nc,
                            ops[:],
                            zt_sb[p][:, st * P : (st + 1) * P],
                            wo_sb[:, p, nn * QB : (nn + 1) * QB],
                            start=(p == 0),
                            stop=(p == 1),
                            fp32r=fp32r,
                        )
                    ot = ost.tile([P, QB], F32, name="ot", tag="ot")
                    nc.vector.tensor_copy(ot[:], ops[:])
                    nc.sync.dma_start(
                        out=out_d[st * P : (st + 1) * P, nn * QB : (nn + 1) * QB],
                        in_=ot[:],
                    )

    nc.compile()
    _PROGRAM_CACHE[mm_dtype] = nc
    return nc


def _round_fp32r(a):
    """Round fp32 array to nearest fp32r (drop 12 low mantissa bits)."""
    u = np.ascontiguousarray(a, dtype=np.float32).view(np.uint32)
    u = ((u + 0x800) & np.uint32(0xFFFFF000)).astype(np.uint32)
    return u.view(np.float32)


def make_in_maps(
    normalized_resid_pre, W_Q, W_K, W_V, W_O, b_Q, b_K, b_V, b_O,
    mm_dtype=MM_DTYPE,
):
    """Shard + prearrange the full inputs into per-core input maps."""
    np_dt = np.float32 if mm_dtype != "bf16" else np.dtype("bfloat16")
    import ml_dtypes  # noqa: F401  (registers bfloat16 with numpy)

    rnd = _round_fp32r if mm_dtype == "fp32r" else (lambda a: a)

    x = np.asarray(normalized_resid_pre, dtype=np.float32)
    W_Q = np.asarray(W_Q, dtype=np.float32)
    W_K = np.asarray(W_K, dtype=np.float32)
    W_V = np.asarray(W_V, dtype=np.float32)
    W_O = np.asarray(W_O, dtype=np.float32)
    b_Q = np.asarray(b_Q, dtype=np.float32)
    b_K = np.asarray(b_K, dtype=np.float32)
    b_V = np.asarray(b_V, dtype=np.float32)

    xT = [rnd(np.ascontiguousarray(x[b].T)).astype(np_dt) for b in range(B)]
    diag = np.triu(np.ones((P, P), dtype=np.float32), k=0).astype(np_dt)

    in_maps = []
    for c in range(NCORES):
        b = c // (NCORES // B)
        heads = [HPC * (c % (NCORES // B)) + i for i in range(HPC)]
        wq = rnd(np.concatenate([W_Q[h] for h in heads], axis=1)).astype(np_dt)
        wk = rnd(np.concatenate([W_K[h] for h in heads], axis=1)).astype(np_dt)
        wv = rnd(np.concatenate([W_V[h] for h in heads], axis=1)).astype(np_dt)
        wo = rnd(np.concatenate([W_O[h] for h in heads], axis=0)).astype(np_dt)
        bq = np.stack(
            [
                np.concatenate([b_Q[heads[0]], b_Q[heads[1]]]),
                np.concatenate([b_Q[heads[2]], b_Q[heads[3]]]),
            ]
        ).astype(np.float32)
        bk = np.stack(
            [
                np.concatenate([b_K[heads[0]], b_K[heads[1]]]),
                np.concatenate([b_K[heads[2]], b_K[heads[3]]]),
            ]
        ).astype(np.float32)
        bv = np.tile(
            np.concatenate([b_V[h] for h in heads])[None, :], (P, 1)
        ).astype(np.float32)
        in_maps.append(
            {
                "xT": np.ascontiguousarray(xT[b]),
                "wq": wq, "wk": wk, "wv": wv, "wo": wo,
                "bq": bq, "bk": bk, "bv": bv,
                "diag": diag,
            }
        )
    return in_maps


def kernel(normalized_resid_pre, W_Q, W_K, W_V, W_O, b_Q, b_K, b_V, b_O):
    global LAST_RESULTS
    nc = build_program()
    in_maps = make_in_maps(
        normalized_resid_pre, W_Q, W_K, W_V, W_O, b_Q, b_K, b_V, b_O
    )
    trace = os.environ.get("ATTN_TRACE", "0") == "1"
    res = run_bass_kernel_spmd(nc, in_maps, list(range(NCORES)), trace=trace)
    LAST_RESULTS = res

    b_O = np.asarray(b_O, dtype=np.float32)
    parts = [np.asarray(res.results[c]["out"], dtype=np.float64) for c in range(NCORES)]
    npc = NCORES // B  # cores per batch
    out = np.stack(
        [sum(parts[b * npc : (b + 1) * npc]) + b_O for b in range(B)]
    )
    return out.astype(np.float32)
